# revision 1
# baseline (speedup 1.0000x reference)
"""Grouped GEMM (MoE expert layers) on 8 Trainium2 NeuronCores.

Problem: output[s_e:e_e] = input[s_e:e_e] @ weight[e].T for 8 experts with
token counts given by expert_offsets; input [16384, 2048] f32,
weight [8, 5632, 2048] f32.

Strategy: tensor-parallel over out_features. Core c computes ALL tokens
against its contiguous 704-wide slice of OUT. The expert segmentation enters
the program only as trace-time loop bounds, which are identical on every
core, so one SPMD program serves all 8 cores. The host pre-transposes x
(-> [IN, T]) and the per-core weight slice (-> [E, IN, 704]) so every DMA is
a natural-layout strided read, and un-shards by concatenating the per-core
[T, 704] outputs along the feature axis.

Matmuls run in float32r (full-rate fp32 streaming on the PE; ~1.5e-4 rel
err for K=2048, vs 4x slower exact float32).
"""
import numpy as np

E, IN, OUT, T, NCORES = 8, 2048, 5632, 16384, 8
OUT_C = OUT // NCORES          # 704 out-features per core
P = 128                        # partitions
KT = IN // P                   # 16 k-tiles of 128
NSPLIT = 352                   # psum bank-sized halves of OUT_C
TT_CHUNK = 2                   # token tiles (128 tokens) per x DMA


def _pad_segments(offsets):
    """Per-expert token counts padded to multiples of P.

    Returns (sizes, padded_sizes, pad_total).
    """
    sizes = np.diff(offsets).astype(int)
    padded = [(-(-s // P)) * P for s in sizes]
    return list(sizes), padded, int(sum(padded))


def _build_program(padded_sizes, dt_in):
    import concourse.bass as bass
    import concourse.mybir as mybir
    from concourse.tile import TileContext
    from wait_legalize_embed import legalize_waits

    Tp = sum(padded_sizes)
    nc = bass.Bass()
    xT_d = nc.dram_tensor("xT", [IN, Tp], dt_in, kind="ExternalInput")
    wT_d = nc.dram_tensor("wT", [E, IN, OUT_C], dt_in, kind="ExternalInput")
    out_d = nc.dram_tensor("out", [Tp, OUT_C], mybir.dt.float32, kind="ExternalOutput")

    xT_r = xT_d.rearrange("(kt p) t -> p kt t", p=P)

    with TileContext(nc) as tc:
        with tc.tile_pool(name="wpool", bufs=2) as wpool, \
             tc.tile_pool(name="xpool", bufs=4) as xpool, \
             tc.tile_pool(name="opool", bufs=4) as opool, \
             tc.tile_pool(name="ppool", bufs=8, space="PSUM") as ppool:
            tile_base = 0
            for e in range(E):
                ntiles = padded_sizes[e] // P
                if ntiles == 0:
                    continue
                w_sb = wpool.tile([P, KT, OUT_C], dt_in, tag="w")
                nc.sync.dma_start(
                    out=w_sb[:], in_=wT_d[e].rearrange("(kt p) n -> p kt n", p=P)
                )
                for tt0 in range(0, ntiles, TT_CHUNK):
                    cur = min(TT_CHUNK, ntiles - tt0)
                    t0 = (tile_base + tt0) * P
                    x_sb = xpool.tile([P, KT, TT_CHUNK * P], dt_in, tag="x")
                    nc.sync.dma_start(
                        out=x_sb[:, :, : cur * P],
                        in_=xT_r[:, :, t0 : t0 + cur * P],
                    )
                    for j in range(cur):
                        ps0 = ppool.tile([P, NSPLIT], mybir.dt.float32, tag="ps")
                        ps1 = ppool.tile([P, NSPLIT], mybir.dt.float32, tag="ps")
                        for kt in range(KT):
                            lhsT = x_sb[:, kt, j * P : (j + 1) * P]
                            nc.tensor.matmul(
                                ps0[:], lhsT, w_sb[:, kt, 0:NSPLIT],
                                start=(kt == 0), stop=(kt == KT - 1),
                            )
                            nc.tensor.matmul(
                                ps1[:], lhsT, w_sb[:, kt, NSPLIT:OUT_C],
                                start=(kt == 0), stop=(kt == KT - 1),
                            )
                        o_sb = opool.tile([P, OUT_C], mybir.dt.float32, tag="o")
                        nc.vector.tensor_copy(o_sb[:, 0:NSPLIT], ps0[:])
                        nc.vector.tensor_copy(o_sb[:, NSPLIT:OUT_C], ps1[:])
                        row = t0 + j * P
                        nc.scalar.dma_start(
                            out=out_d[row : row + P, :], in_=o_sb[:]
                        )
                tile_base += ntiles
    legalize_waits(nc)
    return nc


def _prepare(input, weight, expert_offsets):
    offs = np.asarray(expert_offsets).astype(np.int64)
    sizes, padded_sizes, Tp = _pad_segments(offs)
    x = np.asarray(input, dtype=np.float32)
    w = np.asarray(weight, dtype=np.float32)

    if Tp == T and all(s == p for s, p in zip(sizes, padded_sizes)):
        xT = np.ascontiguousarray(x.T)
    else:
        xp = np.zeros((Tp, IN), dtype=np.float32)
        base = 0
        for e in range(E):
            s, sz = int(offs[e]), sizes[e]
            xp[base : base + sz] = x[s : s + sz]
            base += padded_sizes[e]
        xT = np.ascontiguousarray(xp.T)

    in_maps = []
    for c in range(NCORES):
        wTc = np.ascontiguousarray(
            w[:, c * OUT_C : (c + 1) * OUT_C, :].transpose(0, 2, 1)
        )
        in_maps.append({"xT": xT, "wT": wTc})
    return sizes, padded_sizes, Tp, in_maps


def _gather(results, sizes, padded_sizes):
    full = np.concatenate([r["out"] for r in results], axis=1)
    if sum(sizes) == full.shape[0]:
        return full
    out = np.empty((sum(sizes), OUT), dtype=np.float32)
    base_p = base = 0
    for e in range(E):
        out[base : base + sizes[e]] = full[base_p : base_p + sizes[e]]
        base += sizes[e]
        base_p += padded_sizes[e]
    return out


def run(input, weight, expert_offsets, trace=False):
    import concourse.mybir as mybir
    from concourse.bass_utils import run_bass_kernel_spmd

    sizes, padded_sizes, Tp, in_maps = _prepare(input, weight, expert_offsets)
    nc = _build_program(padded_sizes, mybir.dt.float32r)
    core_ids = list(range(NCORES))
    res = run_bass_kernel_spmd(nc, in_maps, core_ids, trace=trace)
    out = _gather(res.results, sizes, padded_sizes)
    return out, res


def kernel(input, weight, expert_offsets):
    out, _ = run(input, weight, expert_offsets)
    return out


# --- embedded helper (kernel.py must be self-contained) ---------------------
import sys as _sys
import types as _types

_wl_src = '''
import concourse.mybir as mybir


def legalize_waits(nc, maxw: int = 1) -> int:
    """Walrus accepts a limited number of sync-wait commands per instruction;
    split extras onto preceding same-engine NOPs (one wait each)."""
    split = 0
    for f in nc.m.functions:
        for blk in f.blocks:
            new_instructions = []
            for inst in blk.instructions:
                si = inst.sync_info
                waits = list(si.on_wait) if si and si.on_wait else []
                if len(waits) > maxw:
                    keep = waits[-maxw:]
                    extra = waits[:-maxw]
                    for w in extra:
                        nop = mybir.InstNoOp(
                            name=nc.get_next_instruction_name(),
                            sync_info=mybir.SyncInfo(on_wait=[w], on_update=[]),
                            bass_nofuse=True,
                            engine=inst.engine,
                        )
                        new_instructions.append(nop)
                        split += 1
                    inst.sync_info = mybir.SyncInfo(
                        on_wait=keep,
                        on_update=list(si.on_update) if si.on_update else [],
                    )
                new_instructions.append(inst)
            blk.instructions = new_instructions
    return split
'''

_wl_mod = _types.ModuleType("wait_legalize_embed")
exec(_wl_src, _wl_mod.__dict__)
_sys.modules["wait_legalize_embed"] = _wl_mod
